# revision 28
# baseline (speedup 1.0000x reference)
"""Trainium2 Bass kernel for the DLI loss (ragged segment means -> pairwise NLL).

Math reduction: see _host_finish. Heavy work = ragged segment SUM of
encoder_output as a masked matmul seg[T,D] = M[S,T]^T @ x[S,D], data-parallel
over 8 cores (4 batches each).

Pipeline notes (from trace analysis):
- HBM-DMA bound: 4 MB tiles (32 KB/partition descriptors) stream at ~410 GB/s;
  smaller descriptors degrade sharply (1 MB -> 341 GB/s). Batches 0-2 use two
  4 MB tiles; batch 3 tapers 16/8/4/4 chunks so the post-stream backlog is
  short. The taper tiles get dedicated SBUF slots so their DMA triggers never
  wait on casts.
- The x-stream triggers (Sync engine) must never sit behind a trigger with a
  late-satisfied wait, and the cast engines must never wait on late work:
  masks are computed upfront, the dots run AFTER all casts at the end of the
  DVE program (all four psum banks stay live), and batch 3's raw segment sum
  is evacuated by ACT and shipped to the host (host applies wl/wr), so no
  serial mask->matmul->dots chain couples consecutive batches.
- xb (bf16 cast output) slots are per-piece with bufs=8 (~2 tiles of slack)
  so transient matmul lag cannot back-pressure the casts that pace the
  stream triggers.
- ends/wlr broadcasts ride the ACT HWDGE ring (nc.scalar.dma_start): their
  tiny descriptors would otherwise stall the x FIFO (~6 us for 512 tiny
  descriptors observed).

bf16 matmul operands (mask is exact 0/1); the loss averages 64512 pairs so
bf16 noise washes out to ~3e-7 relative error (measured).
"""

import sys
import os

sys.path.insert(0, "/opt/trn_rl_repo")

_jp = os.environ.get("JAX_PLATFORMS")
if _jp is not None and "axon" not in _jp and "jax" not in sys.modules:
    del os.environ["JAX_PLATFORMS"]

import numpy as np

B, S, D, T = 32, 4096, 512, 64
N_CORES = 8
BPC = B // N_CORES          # batches per core
P = 128                     # SBUF partitions
NCH = S // P                # 32 chunks of [128, D] per batch
CPW = 4                     # chunks per cast piece (pieces alternate ACT/DVE)
RPP = 16                    # max chunks per tile (4 MB)

_PROGRAM_CACHE = {}

# (start_row, chunks, chunk_offset); s = row0 + ch*p + c at [p, c].
STD_TILES = [(0, 16, 0), (2048, 16, 16)]
LAST_TILES = [(0, 16, 0), (2048, 8, 16), (3072, 4, 24), (3584, 4, 28)]


def _build_program():
    from contextlib import ExitStack

    import concourse.bacc as bacc
    import concourse.mybir as mybir
    import concourse.tile as tile

    f32 = mybir.dt.float32
    bf16 = mybir.dt.bfloat16

    nc = bacc.Bacc(
        "TRN2", target_bir_lowering=False, debug=False, enable_asserts=False
    )

    x_d = nc.dram_tensor("x", [BPC, S, D], f32, kind="ExternalInput").ap()
    ends_d = nc.dram_tensor("endsb", [BPC, T], f32, kind="ExternalInput").ap()
    wlr_d = nc.dram_tensor("wlr", [2, D], f32, kind="ExternalInput").ap()
    out_d = nc.dram_tensor("out", [T, BPC - 1, 2], f32, kind="ExternalOutput").ap()
    seg3_d = nc.dram_tensor("seg3", [T, D], f32, kind="ExternalOutput").ap()

    tilings = [STD_TILES] * (BPC - 1) + [LAST_TILES]

    with tile.TileContext(nc) as tc, ExitStack() as ctx:
        singles = ctx.enter_context(tc.tile_pool(name="singles", bufs=1))
        xpool = ctx.enter_context(tc.tile_pool(name="xp", bufs=3))
        bpool = ctx.enter_context(tc.tile_pool(name="bp", bufs=10))
        mpool = ctx.enter_context(tc.tile_pool(name="mp", bufs=1))
        spool = ctx.enter_context(tc.tile_pool(name="sp", bufs=1))
        ppool = ctx.enter_context(tc.tile_pool(name="pp", bufs=1, space="PSUM"))

        dma_list = [(b, t) for b in range(BPC) for t in range(len(tilings[b]))]

        def x_dma(b, t):
            row0, ch, _ = tilings[b][t]
            if b == BPC - 1 and t >= 2:
                # Dedicated slots: taper triggers must not wait on casts.
                xt = xpool.tile([P, ch, D], f32, tag=f"tp{t}", bufs=1)
                nc.sync.dma_start(
                    xt[:],
                    x_d[b][row0 : row0 + ch * P, :].rearrange(
                        "(p c) d -> p c d", c=ch
                    ),
                )
                return xt
            xt = xpool.tile([P, RPP, D], f32, tag="xt")
            nc.sync.dma_start(
                xt[:, :ch, :],
                x_d[b][row0 : row0 + ch * P, :].rearrange("(p c) d -> p c d", c=ch),
            )
            return xt

        # First x tile before any setup work.
        xt_next = x_dma(0, 0)

        # Position index tables (gpsimd). The last batch's first tile
        # matches STD_TILES, so iota3 only stores the taper chunks (16..31).
        iota_t = singles.tile([P, NCH, T], f32, tag="iota_t")
        iota3 = singles.tile([P, NCH - 16, T], f32, tag="iota3")
        for row0, ch, coff in STD_TILES:
            nc.gpsimd.iota(
                iota_t[:, coff : coff + ch, :],
                [[1, ch], [0, T]],
                base=row0,
                channel_multiplier=ch,
                allow_small_or_imprecise_dtypes=True,
            )
        for row0, ch, coff in LAST_TILES[1:]:
            nc.gpsimd.iota(
                iota3[:, coff - 16 : coff - 16 + ch, :],
                [[1, ch], [0, T]],
                base=row0,
                channel_multiplier=ch,
                allow_small_or_imprecise_dtypes=True,
            )

        # ends (one broadcast) + wlr on the ACT HWDGE ring: keeps their tiny
        # descriptors out of the x-stream FIFO; the triggers sit at the top
        # of the ACT program with no wait conditions.
        ends_t = singles.tile([P, BPC, T], f32)
        nc.scalar.dma_start(
            ends_t[:], ends_d.unsqueeze(0).to_broadcast((P, BPC, T))
        )
        wlr_t = singles.tile([T, 2, D], f32)
        nc.scalar.dma_start(wlr_t[:], wlr_d.unsqueeze(0).to_broadcast((T, 2, D)))

        out_t = singles.tile([T, BPC - 1, 2], f32)
        seg3_t = singles.tile([T, D], f32)

        # mask[p,i,t] = (s <= end_t) - (s <= end_{t-1}) in {0,1}, bf16.
        # mask(b0) is emitted upfront; the rest are interleaved after the
        # first tiles' casts so they never monopolize DVE (4 back-to-back
        # masks would delay the casts that pace the xt-slot recycle and
        # thus the stream triggers).
        def emit_mask(b):
            cmpe = mpool.tile([P, NCH, T], bf16, tag="cmpe")
            mask = mpool.tile([P, NCH, T], bf16, tag=f"mask{b}")
            if b == BPC - 1:
                nc.vector.tensor_tensor(
                    cmpe[:, :16, :],
                    iota_t[:, :16, :],
                    ends_t[:, b : b + 1, :].to_broadcast((P, 16, T)),
                    op=mybir.AluOpType.is_le,
                )
                nc.vector.tensor_tensor(
                    cmpe[:, 16:, :],
                    iota3[:],
                    ends_t[:, b : b + 1, :].to_broadcast((P, NCH - 16, T)),
                    op=mybir.AluOpType.is_le,
                )
            else:
                nc.vector.tensor_tensor(
                    cmpe[:],
                    iota_t[:],
                    ends_t[:, b : b + 1, :].to_broadcast((P, NCH, T)),
                    op=mybir.AluOpType.is_le,
                )
            nc.vector.tensor_sub(
                mask[:, :, 1:], cmpe[:, :, 1:], cmpe[:, :, : T - 1]
            )
            nc.vector.tensor_copy(mask[:, :, 0:1], cmpe[:, :, 0:1])
            return mask

        masks = [emit_mask(0)]

        psums = []
        tile_counter = 0
        dma_iter = iter(dma_list[1:])
        n_pieces_total = sum(
            (ch + CPW - 1) // CPW for tl in tilings for _, ch, _ in tl
        )
        piece_idx = 0
        for b in range(BPC):
            mask = None
            psum = ppool.tile([T, D], f32, tag=f"ps{b}")
            psums.append(psum)
            for t, (row0, ch, coff) in enumerate(tilings[b]):
                xt = xt_next
                nxt = next(dma_iter, None)
                if nxt is not None:
                    xt_next = x_dma(*nxt)
                npieces = (ch + CPW - 1) // CPW
                mask = masks[b]
                for q in range(npieces):
                    sl = slice(q * CPW, min((q + 1) * CPW, ch))
                    pw = sl.stop - sl.start
                    use_act = piece_idx % 2 == 0
                    if piece_idx == n_pieces_total - 1:
                        use_act = False       # last piece on the faster DVE
                    elif piece_idx == n_pieces_total - 2:
                        use_act = True        # ...in parallel with ACT
                    piece_idx += 1
                    xb = bpool.tile([P, CPW, D], bf16, tag="xb")
                    eng = nc.scalar.copy if use_act else nc.vector.tensor_copy
                    if piece_idx == n_pieces_total:
                        # Final piece: per-chunk cast+matmul interleave so
                        # each matmul starts as soon as its chunk is cast.
                        for c in range(pw):
                            eng(
                                xb[:, c : c + 1, :],
                                xt[:, sl.start + c : sl.start + c + 1, :],
                            )
                            i = coff + sl.start + c
                            nc.tensor.matmul(
                                psum[:],
                                mask[:, i, :],
                                xb[:, c, :],
                                start=(i == 0),
                                stop=(i == NCH - 1),
                            )
                        continue
                    eng(xb[:, :pw, :], xt[:, sl, :])
                    for c in range(pw):
                        i = coff + sl.start + c
                        nc.tensor.matmul(
                            psum[:],
                            mask[:, i, :],
                            xb[:, c, :],
                            start=(i == 0),
                            stop=(i == NCH - 1),
                        )
                tile_counter += 1
                if tile_counter <= BPC - 1:
                    masks.append(emit_mask(tile_counter))

        # Dots for batches 0..2 at the END of the DVE program — they never
        # block casts. All psum banks stay live (4 of 8 banks used).
        for b in range(BPC - 1):
            for d_ in range(2):
                scratch = spool.tile([T, D], f32, tag=f"scr{d_}")
                nc.vector.tensor_mul(scratch[:], psums[b][:], wlr_t[:, d_, :])
                nc.vector.reduce_sum(
                    out_t[:, b, d_ : d_ + 1],
                    scratch[:],
                    axis=mybir.AxisListType.X,
                )
        # Batch 3: raw segment sums evacuated by ACT; host applies wl/wr.
        nc.scalar.copy(seg3_t[:], psums[BPC - 1][:])

        # Output triggers after every x trigger in Sync program order.
        nc.sync.dma_start(out_d[:], out_t[:])
        nc.sync.dma_start(seg3_d[:], seg3_t[:])

    nc.compile()
    return nc


def _host_prep(encoder_output, W, b, his_turn_end_ids):
    x = np.ascontiguousarray(np.asarray(encoder_output, dtype=np.float32))
    W = np.asarray(W, dtype=np.float32)
    bias = np.asarray(b, dtype=np.float32)
    ends = np.asarray(his_turn_end_ids).astype(np.int64)

    ends_prev = np.concatenate(
        [np.full((B, 1), -1, np.int64), ends[:, :-1]], axis=1
    )
    endsb = ends.astype(np.float32)  # [B, T]

    wlr = np.stack([W[:D, 1] - W[:D, 0], W[D:, 1] - W[D:, 0]], axis=0)  # [2, D]
    wlr = np.ascontiguousarray(wlr, dtype=np.float32)
    bd = np.float64(np.float32(bias[1]) - np.float32(bias[0]))

    counts = (ends - ends_prev).astype(np.float64)  # [B, T]
    return x, endsb, wlr, bd, counts


def _host_finish(A0, C0, counts, bd):
    A = A0.astype(np.float64) / counts
    C = C0.astype(np.float64) / counts
    u = A[:, :, None] + C[:, None, :] + bd  # [B, T, T]
    j = np.arange(T)[:, None]
    k = np.arange(T)[None, :]
    tri = k < j
    adj = k == (j - 1)
    nll = np.where(adj, np.logaddexp(0.0, -u), np.logaddexp(0.0, u))
    n_pairs = B * (T * (T - 1) // 2)
    loss = np.sum(np.where(tri, nll, 0.0)) / n_pairs
    return np.asarray(loss, dtype=np.float32)


def kernel(encoder_output, W, b, his_turn_end_ids):
    from concourse.bass_utils import run_bass_kernel_spmd

    x, endsb, wlr, bd, counts = _host_prep(encoder_output, W, b, his_turn_end_ids)

    if "nc" not in _PROGRAM_CACHE:
        _PROGRAM_CACHE["nc"] = _build_program()
    nc = _PROGRAM_CACHE["nc"]

    in_maps = [
        {
            "x": x[i * BPC : (i + 1) * BPC],
            "endsb": endsb[i * BPC : (i + 1) * BPC],
            "wlr": wlr,
        }
        for i in range(N_CORES)
    ]
    trace = bool(int(os.environ.get("BASS_KERNEL_TRACE", "0")))
    kw = {}
    if os.environ.get("BASS_KERNEL_TMPDIR"):
        kw["tmpdir"] = os.environ["BASS_KERNEL_TMPDIR"]
    res = run_bass_kernel_spmd(nc, in_maps, list(range(N_CORES)), trace=trace, **kw)
    _PROGRAM_CACHE["last_results"] = res

    A0 = np.empty((B, T), np.float64)
    C0 = np.empty((B, T), np.float64)
    for i, r in enumerate(res.results):
        dots = r["out"]  # [T, BPC-1, 2]
        seg3 = r["seg3"].astype(np.float64)  # [T, D]
        for j in range(BPC - 1):
            A0[i * BPC + j] = dots[:, j, 0]
            C0[i * BPC + j] = dots[:, j, 1]
        A0[i * BPC + BPC - 1] = seg3 @ wlr[0].astype(np.float64)
        C0[i * BPC + BPC - 1] = seg3 @ wlr[1].astype(np.float64)
    return _host_finish(A0, C0, counts, bd)


# revision 29
# speedup vs baseline: 1.0128x; 1.0128x over previous
"""Trainium2 Bass kernel for the DLI loss (ragged segment means -> pairwise NLL).

Math reduction: see _host_finish. Heavy work = ragged segment SUM of
encoder_output as a masked matmul seg[T,D] = M[S,T]^T @ x[S,D], data-parallel
over 8 cores (4 batches each).

Pipeline notes (from trace analysis):
- HBM-DMA bound: 4 MB tiles (32 KB/partition descriptors) stream at ~410 GB/s;
  smaller descriptors degrade sharply (1 MB -> 341 GB/s). Batches 0-2 use two
  4 MB tiles; batch 3 tapers 16/8/4/4 chunks so the post-stream backlog is
  short. The taper tiles get dedicated SBUF slots so their DMA triggers never
  wait on casts.
- The x-stream triggers (Sync engine) must never sit behind a trigger with a
  late-satisfied wait, and the cast engines must never wait on late work:
  masks are computed upfront, the dots run AFTER all casts at the end of the
  DVE program (all four psum banks stay live), and batch 3's raw segment sum
  is evacuated by ACT and shipped to the host (host applies wl/wr), so no
  serial mask->matmul->dots chain couples consecutive batches.
- xb (bf16 cast output) slots are per-piece with bufs=8 (~2 tiles of slack)
  so transient matmul lag cannot back-pressure the casts that pace the
  stream triggers.
- ends/wlr broadcasts ride the ACT HWDGE ring (nc.scalar.dma_start): their
  tiny descriptors would otherwise stall the x FIFO (~6 us for 512 tiny
  descriptors observed).

bf16 matmul operands (mask is exact 0/1); the loss averages 64512 pairs so
bf16 noise washes out to ~3e-7 relative error (measured).
"""

import sys
import os

sys.path.insert(0, "/opt/trn_rl_repo")

_jp = os.environ.get("JAX_PLATFORMS")
if _jp is not None and "axon" not in _jp and "jax" not in sys.modules:
    del os.environ["JAX_PLATFORMS"]

import numpy as np

B, S, D, T = 32, 4096, 512, 64
N_CORES = 8
BPC = B // N_CORES          # batches per core
P = 128                     # SBUF partitions
NCH = S // P                # 32 chunks of [128, D] per batch
CPW = 4                     # chunks per cast piece (pieces alternate ACT/DVE)
RPP = 16                    # max chunks per tile (4 MB)

_PROGRAM_CACHE = {}

# (start_row, chunks, chunk_offset); s = row0 + ch*p + c at [p, c].
STD_TILES = [(0, 16, 0), (2048, 16, 16)]
LAST_TILES = [(0, 16, 0), (2048, 8, 16), (3072, 4, 24), (3584, 4, 28)]


def _build_program():
    from contextlib import ExitStack

    import concourse.bacc as bacc
    import concourse.mybir as mybir
    import concourse.tile as tile

    f32 = mybir.dt.float32
    bf16 = mybir.dt.bfloat16

    nc = bacc.Bacc(
        "TRN2", target_bir_lowering=False, debug=False, enable_asserts=False
    )

    x_d = nc.dram_tensor("x", [BPC, S, D], f32, kind="ExternalInput").ap()
    ends_d = nc.dram_tensor("endsb", [BPC, T], f32, kind="ExternalInput").ap()
    wlr_d = nc.dram_tensor("wlr", [2, D], f32, kind="ExternalInput").ap()
    out_d = nc.dram_tensor("out", [T, BPC - 1, 2], f32, kind="ExternalOutput").ap()
    seg3_d = nc.dram_tensor("seg3", [T, D], f32, kind="ExternalOutput").ap()

    tilings = [STD_TILES] * (BPC - 1) + [LAST_TILES]

    with tile.TileContext(nc) as tc, ExitStack() as ctx:
        singles = ctx.enter_context(tc.tile_pool(name="singles", bufs=1))
        xpool = ctx.enter_context(tc.tile_pool(name="xp", bufs=3))
        bpool = ctx.enter_context(tc.tile_pool(name="bp", bufs=10))
        mpool = ctx.enter_context(tc.tile_pool(name="mp", bufs=1))
        spool = ctx.enter_context(tc.tile_pool(name="sp", bufs=1))
        ppool = ctx.enter_context(tc.tile_pool(name="pp", bufs=1, space="PSUM"))

        dma_list = [(b, t) for b in range(BPC) for t in range(len(tilings[b]))]

        def x_dma(b, t):
            row0, ch, _ = tilings[b][t]
            if b == BPC - 1 and t >= 2:
                # Dedicated slots: taper triggers must not wait on casts.
                xt = xpool.tile([P, ch, D], f32, tag=f"tp{t}", bufs=1)
                nc.sync.dma_start(
                    xt[:],
                    x_d[b][row0 : row0 + ch * P, :].rearrange(
                        "(p c) d -> p c d", c=ch
                    ),
                )
                return xt
            xt = xpool.tile([P, RPP, D], f32, tag="xt")
            nc.sync.dma_start(
                xt[:, :ch, :],
                x_d[b][row0 : row0 + ch * P, :].rearrange("(p c) d -> p c d", c=ch),
            )
            return xt

        # First x tile before any setup work.
        xt_next = x_dma(0, 0)

        # Position index tables (gpsimd). The last batch's first tile
        # matches STD_TILES, so iota3 only stores the taper chunks (16..31).
        iota_t = singles.tile([P, NCH, T], f32, tag="iota_t")
        iota3 = singles.tile([P, NCH - 16, T], f32, tag="iota3")
        for row0, ch, coff in STD_TILES:
            nc.gpsimd.iota(
                iota_t[:, coff : coff + ch, :],
                [[1, ch], [0, T]],
                base=row0,
                channel_multiplier=ch,
                allow_small_or_imprecise_dtypes=True,
            )
        for row0, ch, coff in LAST_TILES[1:]:
            nc.gpsimd.iota(
                iota3[:, coff - 16 : coff - 16 + ch, :],
                [[1, ch], [0, T]],
                base=row0,
                channel_multiplier=ch,
                allow_small_or_imprecise_dtypes=True,
            )

        # ends (one broadcast) + wlr on the ACT HWDGE ring: keeps their tiny
        # descriptors out of the x-stream FIFO; the triggers sit at the top
        # of the ACT program with no wait conditions.
        ends_t = singles.tile([P, BPC, T], f32)
        nc.scalar.dma_start(
            ends_t[:], ends_d.unsqueeze(0).to_broadcast((P, BPC, T))
        )
        wlr_t = singles.tile([T, 2, D], f32)
        nc.scalar.dma_start(wlr_t[:], wlr_d.unsqueeze(0).to_broadcast((T, 2, D)))

        out_t = singles.tile([T, BPC - 1, 2], f32)
        seg3_t = singles.tile([T, D], f32)

        # mask[p,i,t] = (s <= end_t) - (s <= end_{t-1}) in {0,1}, bf16.
        # mask(b0) is emitted upfront; the rest are interleaved after the
        # first tiles' casts so they never monopolize DVE (4 back-to-back
        # masks would delay the casts that pace the xt-slot recycle and
        # thus the stream triggers).
        def emit_mask(b):
            cmpe = mpool.tile([P, NCH, T], bf16, tag="cmpe")
            mask = mpool.tile([P, NCH, T], bf16, tag=f"mask{b}")
            if b == BPC - 1:
                nc.vector.tensor_tensor(
                    cmpe[:, :16, :],
                    iota_t[:, :16, :],
                    ends_t[:, b : b + 1, :].to_broadcast((P, 16, T)),
                    op=mybir.AluOpType.is_le,
                )
                nc.vector.tensor_tensor(
                    cmpe[:, 16:, :],
                    iota3[:],
                    ends_t[:, b : b + 1, :].to_broadcast((P, NCH - 16, T)),
                    op=mybir.AluOpType.is_le,
                )
            else:
                nc.vector.tensor_tensor(
                    cmpe[:],
                    iota_t[:],
                    ends_t[:, b : b + 1, :].to_broadcast((P, NCH, T)),
                    op=mybir.AluOpType.is_le,
                )
            nc.vector.tensor_sub(
                mask[:, :, 1:], cmpe[:, :, 1:], cmpe[:, :, : T - 1]
            )
            nc.vector.tensor_copy(mask[:, :, 0:1], cmpe[:, :, 0:1])
            return mask

        masks = [emit_mask(0)]

        psums = []
        tile_counter = 0
        dma_iter = iter(dma_list[1:])
        n_pieces_total = sum(
            (ch + CPW - 1) // CPW for tl in tilings for _, ch, _ in tl
        )
        piece_idx = 0
        for b in range(BPC):
            mask = None
            psum = ppool.tile([T, D], f32, tag=f"ps{b}")
            psums.append(psum)
            for t, (row0, ch, coff) in enumerate(tilings[b]):
                xt = xt_next
                nxt = next(dma_iter, None)
                if nxt is not None:
                    xt_next = x_dma(*nxt)
                npieces = (ch + CPW - 1) // CPW
                mask = masks[b]
                for q in range(npieces):
                    sl = slice(q * CPW, min((q + 1) * CPW, ch))
                    pw = sl.stop - sl.start
                    use_act = piece_idx % 2 == 0
                    if piece_idx == n_pieces_total - 1:
                        use_act = False       # last piece on the faster DVE
                    elif piece_idx == n_pieces_total - 2:
                        use_act = True        # ...in parallel with ACT
                    piece_idx += 1
                    xb = bpool.tile([P, CPW, D], bf16, tag="xb")
                    eng = nc.scalar.copy if use_act else nc.vector.tensor_copy
                    eng(xb[:, :pw, :], xt[:, sl, :])
                    for c in range(pw):
                        i = coff + sl.start + c
                        nc.tensor.matmul(
                            psum[:],
                            mask[:, i, :],
                            xb[:, c, :],
                            start=(i == 0),
                            stop=(i == NCH - 1),
                        )
                tile_counter += 1
                if tile_counter <= BPC - 1:
                    masks.append(emit_mask(tile_counter))

        # Dots for batches 0..2 at the END of the DVE program — they never
        # block casts. All psum banks stay live (4 of 8 banks used).
        for b in range(BPC - 1):
            for d_ in range(2):
                scratch = spool.tile([T, D], f32, tag=f"scr{d_}")
                nc.vector.tensor_mul(scratch[:], psums[b][:], wlr_t[:, d_, :])
                nc.vector.reduce_sum(
                    out_t[:, b, d_ : d_ + 1],
                    scratch[:],
                    axis=mybir.AxisListType.X,
                )
        # Batch 3: raw segment sums evacuated by ACT; host applies wl/wr.
        nc.scalar.copy(seg3_t[:], psums[BPC - 1][:])

        # Output triggers after every x trigger in Sync program order.
        nc.sync.dma_start(out_d[:], out_t[:])
        nc.sync.dma_start(seg3_d[:], seg3_t[:])

    nc.compile()
    return nc


def _host_prep(encoder_output, W, b, his_turn_end_ids):
    x = np.ascontiguousarray(np.asarray(encoder_output, dtype=np.float32))
    W = np.asarray(W, dtype=np.float32)
    bias = np.asarray(b, dtype=np.float32)
    ends = np.asarray(his_turn_end_ids).astype(np.int64)

    ends_prev = np.concatenate(
        [np.full((B, 1), -1, np.int64), ends[:, :-1]], axis=1
    )
    endsb = ends.astype(np.float32)  # [B, T]

    wlr = np.stack([W[:D, 1] - W[:D, 0], W[D:, 1] - W[D:, 0]], axis=0)  # [2, D]
    wlr = np.ascontiguousarray(wlr, dtype=np.float32)
    bd = np.float64(np.float32(bias[1]) - np.float32(bias[0]))

    counts = (ends - ends_prev).astype(np.float64)  # [B, T]
    return x, endsb, wlr, bd, counts


def _host_finish(A0, C0, counts, bd):
    A = A0.astype(np.float64) / counts
    C = C0.astype(np.float64) / counts
    u = A[:, :, None] + C[:, None, :] + bd  # [B, T, T]
    j = np.arange(T)[:, None]
    k = np.arange(T)[None, :]
    tri = k < j
    adj = k == (j - 1)
    nll = np.where(adj, np.logaddexp(0.0, -u), np.logaddexp(0.0, u))
    n_pairs = B * (T * (T - 1) // 2)
    loss = np.sum(np.where(tri, nll, 0.0)) / n_pairs
    return np.asarray(loss, dtype=np.float32)


def kernel(encoder_output, W, b, his_turn_end_ids):
    from concourse.bass_utils import run_bass_kernel_spmd

    x, endsb, wlr, bd, counts = _host_prep(encoder_output, W, b, his_turn_end_ids)

    if "nc" not in _PROGRAM_CACHE:
        _PROGRAM_CACHE["nc"] = _build_program()
    nc = _PROGRAM_CACHE["nc"]

    in_maps = [
        {
            "x": x[i * BPC : (i + 1) * BPC],
            "endsb": endsb[i * BPC : (i + 1) * BPC],
            "wlr": wlr,
        }
        for i in range(N_CORES)
    ]
    trace = bool(int(os.environ.get("BASS_KERNEL_TRACE", "0")))
    kw = {}
    if os.environ.get("BASS_KERNEL_TMPDIR"):
        kw["tmpdir"] = os.environ["BASS_KERNEL_TMPDIR"]
    res = run_bass_kernel_spmd(nc, in_maps, list(range(N_CORES)), trace=trace, **kw)
    _PROGRAM_CACHE["last_results"] = res

    A0 = np.empty((B, T), np.float64)
    C0 = np.empty((B, T), np.float64)
    for i, r in enumerate(res.results):
        dots = r["out"]  # [T, BPC-1, 2]
        seg3 = r["seg3"].astype(np.float64)  # [T, D]
        for j in range(BPC - 1):
            A0[i * BPC + j] = dots[:, j, 0]
            C0[i * BPC + j] = dots[:, j, 1]
        A0[i * BPC + BPC - 1] = seg3 @ wlr[0].astype(np.float64)
        C0[i * BPC + BPC - 1] = seg3 @ wlr[1].astype(np.float64)
    return _host_finish(A0, C0, counts, bd)
